# revision 49
# baseline (speedup 1.0000x reference)
"""Trainium2 Bass kernel for nn_ListenerModel (scatter_memory).

Data-parallel over batch (B=64 -> 8 rows/core):
 - bf16 matmul operands everywhere (half the HBM bytes of fp32/fp32r at
   the same PE rate, no small-free-dim penalty).
 - Host-side compaction of masked softmax positions: ~50% of L=512 get
   weight exactly 0, so the mm1->mm2->att chain runs on LC=288 columns.
 - W_vis stays replicated (a ReduceScatter variant was tried: the
   collective runway is ~65us from kernel start and couples the span to
   inter-core launch skew, which is both slower in expectation and high
   variance).  The 12.6MB bf16 stream is consumed chunk-by-chunk by the
   PE, interleaved with mm1 so the array never waits for it.
 - Partition-major packed DRAM layouts (multi-KB DMA descriptor lines),
   latency-ordered on the Sync queue; tiny constants on GpSimd.
 - Softmax without max-subtraction (scores are tanh-bounded, exp is
   fp32-safe); the mask row (carrying b_a2) is accumulated into the
   score matmul as a rank-1 PE term; softmax scale on DVE.
 - The softmax -> broadcast -> weighted-sum tail is software-pipelined
   one batch row behind the mm2/mm3 chain, and the final
   sep_fin . attended dot is a per-row PE matmul (sepfinT x attT col).
"""

import numpy as np
import ml_dtypes
from contextlib import ExitStack

import concourse.bass as bass
import concourse.mybir as mybir
from concourse import bacc, tile
from concourse.bass_utils import run_bass_kernel_spmd

NCORES = 8
B, L, S, H = 64, 512, 6, 8
EMBED, HID, IMG, ATT = 1024, 512, 2048, 256
SIMG = S * IMG          # 12288
BC = B // NCORES        # 8 batch rows per core
BS = BC * S             # 48 (b,s) rows per core
BSH = BS * H            # 384
P = 128
LC = 288                # compacted sequence length (max valid ~284)
FP = mybir.dt.float32
FPR = mybir.dt.float32r
BF = mybir.dt.bfloat16

KE = EMBED // P         # 8  k-chunks for EMBED contraction
KH = HID // P           # 4  k-chunks for HID contraction
KA = ATT // P           # 2  k-chunks for ATT contraction
KI = IMG // P           # 16 k-chunks for separate-image projection
KBH = BSH // P          # 3  k-chunks for history averaging
KV = SIMG // P          # 96 k-chunks for the visual-context matmul
NVG = 12                # W_vis DMA groups
KVG = KV // NVG         # 8 k-chunks per group
NHT = HID // P          # 4  hid tiles
NAT = ATT // P          # 2  att tiles

# packed [P, *] fp32 constants: ident | bembc | bvisc | bmmc | ba1c
CPW = P + NHT * 3 + NAT
# packed [1, *] bf16: ones_bf | bsep_row | bemb_row | mask rows
C1BW = P + HID + HID + BC * LC

bf16 = ml_dtypes.bfloat16


def build_nc():
    nc = bacc.Bacc(None)

    d_constP = nc.dram_tensor("constP", [P, CPW], FP, kind="ExternalInput")
    d_const1b = nc.dram_tensor("const1b", [1, C1BW], BF, kind="ExternalInput")
    d_reps = nc.dram_tensor("reps8", [BC, P, KE * LC], BF, kind="ExternalInput")
    d_wvis = nc.dram_tensor("wvis8", [NVG, P, KVG * HID], BF,
                            kind="ExternalInput")
    d_vct = nc.dram_tensor("vct8", [P, KV * BC], BF, kind="ExternalInput")
    d_wemb = nc.dram_tensor("wemb8", [P, KE * HID], BF, kind="ExternalInput")
    d_wmm = nc.dram_tensor("wmm8", [P, 2 * KH * HID], BF, kind="ExternalInput")
    d_wa1 = nc.dram_tensor("wa18", [P, KH * ATT], BF, kind="ExternalInput")
    d_wsep = nc.dram_tensor("wsep8", [P, KI * HID], BF, kind="ExternalInput")
    d_wa2 = nc.dram_tensor("wa28", [P, KA], BF, kind="ExternalInput")
    d_sepT = nc.dram_tensor("sepT8", [P, KI * BS], BF, kind="ExternalInput")
    d_hist = nc.dram_tensor("histf8", [P, KBH * EMBED], BF, kind="ExternalInput")
    d_validW = nc.dram_tensor("validW8", [P, KBH * BS], BF, kind="ExternalInput")
    d_hh = nc.dram_tensor("hh_col", [BS, 1], FP, kind="ExternalInput")
    d_out = nc.dram_tensor("out", [S, BC], FP, kind="ExternalOutput")

    AFT = mybir.ActivationFunctionType
    AX = mybir.AxisListType

    with ExitStack() as ctx:
        tc = ctx.enter_context(tile.TileContext(nc))
        wres = ctx.enter_context(tc.tile_pool(name="wres", bufs=1))
        wvp = ctx.enter_context(tc.tile_pool(name="wvp", bufs=3))
        repsp = ctx.enter_context(tc.tile_pool(name="repsp", bufs=8))
        mm1p = ctx.enter_context(tc.tile_pool(name="mm1p", bufs=32))
        mm2p = ctx.enter_context(tc.tile_pool(name="mm2p", bufs=8))
        atthp = ctx.enter_context(tc.tile_pool(name="atthp", bufs=4))
        tmpp = ctx.enter_context(tc.tile_pool(name="tmpp", bufs=2))
        smp = ctx.enter_context(tc.tile_pool(name="smp", bufs=4))
        psA = ctx.enter_context(tc.tile_pool(name="psA", bufs=5, space="PSUM"))
        psB = ctx.enter_context(tc.tile_pool(name="psB", bufs=2, space="PSUM"))
        psD = ctx.enter_context(tc.tile_pool(name="psD", bufs=1, space="PSUM"))

        def wtile(shape, tag, dt=FP):
            return wres.tile(shape, dt, tag=tag, name=tag)

        def body():
            # ---- Sync queue: big loads in latency-priority order ----
            vct = wtile([P, KV * BC], "vct", BF)
            nc.sync.dma_start(out=vct, in_=d_vct[:, :])
            wv_loads = []  # (tile, n_chunks)

            def load_wv(g, c0=0, cn=KVG):
                t = wvp.tile([P, cn * HID], BF, tag="wv", name=f"wv{g}_{c0}")
                nc.sync.dma_start(out=t, in_=d_wvis[g][:, c0 * HID:(c0 + cn) * HID])
                wv_loads.append((t, cn))

            rt = []

            def load_rt(b):
                t = repsp.tile([P, KE * LC], BF, tag="reps", name="rt")
                nc.sync.dma_start(out=t, in_=d_reps[b])
                rt.append(t)

            load_wv(0, 0, 4)
            load_wv(0, 4, 4)
            wemb = wtile([P, KE * HID], "wemb", BF)
            nc.sync.dma_start(out=wemb, in_=d_wemb[:, :])
            load_rt(0)
            load_wv(1)
            load_rt(1)
            load_wv(2)
            load_rt(2)
            load_rt(3)
            load_wv(3)
            for b in range(4, BC):
                load_rt(b)
            wmm = wtile([P, 2 * KH * HID], "wmm", BF)
            nc.sync.dma_start(out=wmm, in_=d_wmm[:, :])
            for g in range(4, NVG):
                load_wv(g)
            wsep = wtile([P, KI * HID], "wsep", BF)
            nc.sync.dma_start(out=wsep, in_=d_wsep[:, :])
            sepT = wtile([P, KI * BS], "sepT", BF)
            nc.sync.dma_start(out=sepT, in_=d_sepT[:, :])
            histf = wtile([P, KBH * EMBED], "histf", BF)
            nc.sync.dma_start(out=histf, in_=d_hist[:, :])
            validW = wtile([P, KBH * BS], "validW", BF)
            nc.sync.dma_start(out=validW, in_=d_validW[:, :])
            wa1 = wtile([P, KH * ATT], "wa1", BF)
            nc.sync.dma_start(out=wa1, in_=d_wa1[:, :])
            wa2_sb = wtile([P, KA], "wa2", BF)
            nc.sync.dma_start(out=wa2_sb, in_=d_wa2[:, :])

            # ---- GpSimd queue: tiny constants ----
            constP = wtile([P, CPW], "constP")
            nc.gpsimd.dma_start(out=constP, in_=d_constP[:, :])
            const1b = wtile([1, C1BW], "const1b", BF)
            nc.gpsimd.dma_start(out=const1b, in_=d_const1b[:, :])
            hh_sb = wtile([BS, 1], "hh")
            nc.gpsimd.dma_start(out=hh_sb, in_=d_hh[:, :])

            ident = constP[:, 0:P]
            bembc = constP[:, P:P + NHT]
            bvisc = constP[:, P + NHT:P + 2 * NHT]
            bmmc = constP[:, P + 2 * NHT:P + 3 * NHT]
            ba1c = constP[:, P + 3 * NHT:P + 3 * NHT + NAT]
            ones_bf = const1b[:, 0:P]
            bsep_row = const1b[:, P:P + HID]
            bemb_row = const1b[:, P + HID:P + 2 * HID]
            mrows = const1b[:, P + 2 * HID:]

            # ---- interleaved: vc chunk-groups + mm1 per batch row ----
            # vc_ps accumulates the whole 96-chunk contraction for this
            # core's 8 rows; mm1T[b] = relu(Wemb.T @ repsT[b] + b_emb)
            vc_ps = psB.tile([BC, HID], FP, tag="B", name="vc_ps")
            mm1_sb = {}

            kcur = 0

            def emit_vc_item(item):
                nonlocal kcur
                wv, cn = item
                for j in range(cn):
                    k = kcur + j
                    nc.tensor.matmul(vc_ps[:, :], vct[:, k * BC:(k + 1) * BC],
                                     wv[:, j * HID:(j + 1) * HID],
                                     start=(k == 0), stop=(k == KV - 1))
                kcur += cn

            def emit_mm1(b):
                for h in range(NHT):
                    ps = psA.tile([P, 512], FP, tag="A", name="mm1ps")
                    for k in range(KE):
                        nc.tensor.matmul(
                            ps[:, :LC],
                            wemb[:, k * HID + h * P:k * HID + (h + 1) * P],
                            rt[b][:, k * LC:(k + 1) * LC],
                            start=(k == 0), stop=(k == KE - 1))
                    t = mm1p.tile([P, LC], BF, tag="mm1", name=f"mm1_{b}_{h}")
                    nc.scalar.activation(t, ps[:, :LC], AFT.Relu,
                                         bias=bembc[:, h:h + 1])
                    mm1_sb[(b, h)] = t

            # ~12 wvis k-chunks per mm1 row keeps the vc stream fed
            idone = 0
            for b in range(BC):
                ktarget = (b + 1) * KV // BC
                while kcur < ktarget and idone < len(wv_loads):
                    emit_vc_item(wv_loads[idone])
                    idone += 1
                emit_mm1(b)
            while idone < len(wv_loads):
                emit_vc_item(wv_loads[idone])
                idone += 1

            # ctx: copy out of PSUM, transpose, relu(+bvis), ctxmm
            ctx_sb = wtile([BC, HID], "ctx_sb")
            nc.scalar.activation(ctx_sb, vc_ps[:, :], AFT.Identity)
            ctxT = []
            for h in range(NHT):
                tp = psB.tile([P, BC], FP, tag="B", name="ctxT_ps")
                nc.tensor.transpose(tp[:, :], ctx_sb[:, h * P:(h + 1) * P],
                                    ident[:BC, :BC])
                t = wtile([P, BC], f"ctxT{h}", BF)
                nc.scalar.activation(t, tp[:, :], AFT.Relu,
                                     bias=bvisc[:, h:h + 1])
                ctxT.append(t)
            ctxmmb = []
            for h2 in range(NHT):
                ps = psB.tile([P, BC], FP, tag="B", name="ctxmm_ps")
                for k in range(KH):
                    nc.tensor.matmul(
                        ps[:, :],
                        wmm[:, (KH + k) * HID + h2 * P:(KH + k) * HID + (h2 + 1) * P],
                        ctxT[k][:, :],
                        start=(k == 0), stop=(k == KH - 1))
                t = wtile([P, BC], f"ctxmmb{h2}")
                nc.scalar.activation(t, ps[:, :], AFT.Identity,
                                     bias=bmmc[:, h2:h2 + 1])
                ctxmmb.append(t)

            # ---- separate images projection: sep[48, 512] ----
            sep_ps = psB.tile([BS, HID], FP, tag="B", name="sep_ps")
            for k in range(KI):
                nc.tensor.matmul(sep_ps[:, :], sepT[:, k * BS:(k + 1) * BS],
                                 wsep[:, k * HID:(k + 1) * HID],
                                 start=(k == 0), stop=False)
            nc.tensor.matmul(sep_ps[:, :], ones_bf[:, :BS], bsep_row,
                             start=False, stop=True)
            sep_sb = wtile([BS, HID], "sep_sb")
            nc.vector.tensor_copy(sep_sb, sep_ps[:, :])

            # ---- history: havgT[e] = histf.T @ validW (pre-averaged) ----
            havgT = []
            for e in range(KE):
                ps = psB.tile([P, BS], FP, tag="B", name="havg_ps")
                for k in range(KBH):
                    nc.tensor.matmul(
                        ps[:, :],
                        histf[:, k * EMBED + e * P:k * EMBED + (e + 1) * P],
                        validW[:, k * BS:(k + 1) * BS],
                        start=(k == 0), stop=(k == KBH - 1))
                t = wtile([P, BS], f"havgT{e}", BF)
                nc.scalar.activation(t, ps[:, :], AFT.Identity)
                havgT.append(t)

            # hist_add[48, 512] = relu(havg @ W_emb + b_emb)
            ha_ps = psB.tile([BS, HID], FP, tag="B", name="ha_ps")
            for e in range(KE):
                nc.tensor.matmul(ha_ps[:, :], havgT[e][:, :],
                                 wemb[:, e * HID:(e + 1) * HID],
                                 start=(e == 0), stop=False)
            nc.tensor.matmul(ha_ps[:, :], ones_bf[:, :BS], bemb_row,
                             start=False, stop=True)
            hadd_sb = wtile([BS, HID], "hadd_sb")
            nc.scalar.activation(hadd_sb, ha_ps[:, :], AFT.Relu)

            # sep_final = sep + hh * hist_add, then transpose to [hid, 48]
            sepfin = wtile([BS, HID], "sepfin")
            nc.vector.tensor_scalar_mul(sepfin, hadd_sb, hh_sb)
            nc.vector.tensor_add(sepfin, sepfin, sep_sb)
            sepfinT = []
            for h in range(NHT):
                tp = psB.tile([P, BS], FP, tag="B", name="sft_ps")
                nc.tensor.transpose(tp[:, :], sepfin[:, h * P:(h + 1) * P],
                                    ident[:BS, :BS])
                t = wtile([P, BS], f"sepfinT{h}")
                nc.scalar.activation(t, tp[:, :], AFT.Identity)
                sepfinT.append(t)

            # ---- per-b chain, softmax/weighted-sum pipelined one b behind ----
            attT = [wtile([P, BC], f"attT{h}") for h in range(NHT)]
            out_sb = wtile([S, BC], "out_sb")
            pend = {}

            def emit_tail(bp):
                wrow, mm2t = pend.pop(bp)
                wb_ps = psA.tile([P, 512], FP, tag="A", name="wbps")
                nc.tensor.matmul(wb_ps[:, :LC], ones_bf, wrow[:, :],
                                 start=True, stop=True)
                for h2 in range(NHT):
                    tmp = tmpp.tile([P, LC], BF, tag="tmpa", name="tmpa")
                    nc.vector.tensor_mul(tmp, mm2t[h2][:, :], wb_ps[:, :LC])
                    nc.vector.reduce_sum(attT[h2][:, bp:bp + 1], tmp,
                                         axis=AX.X)

            def emit_dot(bp):
                # out[6] for this b: sepfinT chunks . attT column (PE dot)
                dps = psD.tile([S, 1], FP, tag="D", name="dot_ps")
                for h in range(NHT):
                    nc.tensor.matmul(dps[:, :],
                                     sepfinT[h][:, bp * S:(bp + 1) * S],
                                     attT[h][:, bp:bp + 1],
                                     start=(h == 0), stop=(h == NHT - 1))
                nc.vector.tensor_copy(out_sb[:, bp:bp + 1], dps[:, :])

            for b in range(BC):
                # mm2 = relu(Wmm_top.T @ mm1 + ctxmm[:, b]) -> bf16.
                # finalize split DVE/ACT via the per-partition ctx bias.
                mm2t = []
                for h2 in range(NHT):
                    ps = psA.tile([P, 512], FP, tag="A", name="mm2ps")
                    for k in range(KH):
                        nc.tensor.matmul(
                            ps[:, :LC],
                            wmm[:, k * HID + h2 * P:k * HID + (h2 + 1) * P],
                            mm1_sb[(b, k)][:, :],
                            start=(k == 0), stop=(k == KH - 1))
                    t = mm2p.tile([P, LC], BF, tag="mm2", name="mm2t")
                    if h2 < 2:
                        nc.vector.tensor_scalar(
                            out=t, in0=ps[:, :LC],
                            scalar1=ctxmmb[h2][:, b:b + 1], scalar2=0.0,
                            op0=mybir.AluOpType.add, op1=mybir.AluOpType.max)
                    else:
                        nc.scalar.activation(t, ps[:, :LC], AFT.Relu,
                                             bias=ctxmmb[h2][:, b:b + 1])
                    mm2t.append(t)
                # mm3: atthT = tanh(W_a1.T @ mm2T + b_a1)
                atth = []
                for a in range(NAT):
                    ps = psA.tile([P, 512], FP, tag="A", name="mm3ps")
                    for k in range(KH):
                        nc.tensor.matmul(
                            ps[:, :LC],
                            wa1[:, k * ATT + a * P:k * ATT + (a + 1) * P],
                            mm2t[k][:, :],
                            start=(k == 0), stop=(k == KH - 1))
                    t = atthp.tile([P, LC], BF, tag="atth", name="atht")
                    nc.scalar.activation(t, ps[:, :LC], AFT.Tanh,
                                         bias=ba1c[:, a:a + 1])
                    atth.append(t)
                # scores row [1, LC] = W_a2.T @ atthT; the mask row (which
                # carries b_a2 too) is accumulated as a rank-1 PE term
                sc_ps = psB.tile([1, 512], FP, tag="B", name="scps")
                for k in range(KA):
                    nc.tensor.matmul(sc_ps[:, :LC], wa2_sb[:, k:k + 1],
                                     atth[k][:, :],
                                     start=(k == 0), stop=False)
                nc.tensor.matmul(sc_ps[:, :LC], ones_bf[:, 0:1],
                                 mrows[:, b * LC:(b + 1) * LC],
                                 start=False, stop=True)
                # softmax without max-subtraction: scores are tanh-bounded
                # (|s| <= 25.6), exp stays comfortably inside fp32
                att_row = smp.tile([1, LC], FP, tag="attrow", name="att_row")
                esum = smp.tile([1, 1], FP, tag="esum", name="esum")
                nc.scalar.activation(att_row, sc_ps[:, :LC], AFT.Exp,
                                     accum_out=esum)
                rec = smp.tile([1, 1], FP, tag="rec", name="rec")
                nc.vector.reciprocal(rec, esum)
                wrow = smp.tile([1, LC], BF, tag="wrow", name="wrow")
                nc.vector.tensor_scalar_mul(wrow, att_row, rec)
                pend[b] = (wrow, mm2t)
                if b > 0:
                    emit_tail(b - 1)
                if b > 1:
                    emit_dot(b - 2)
            emit_tail(BC - 1)
            emit_dot(BC - 2)
            emit_dot(BC - 1)

            nc.sync.dma_start(out=d_out[:, :], in_=out_sb)

        body()

    nc.compile()
    return nc


def _packT(a, np_dt=bf16):
    """[K, W] fp32 -> [P, (K//P)*W] partition-major packed."""
    a = np.ascontiguousarray(a, np.float32)
    K, W = a.shape
    return np.ascontiguousarray(
        a.reshape(K // P, P, W).transpose(1, 0, 2).reshape(P, (K // P) * W)
    ).astype(np_dt)


_NC_CACHE = None


def kernel(reps, separate_imgs, visual_context, masks, hist, hist_len,
           W_vis, b_vis, W_emb, b_emb, W_mm, b_mm, W_sep, b_sep,
           W_a1, b_a1, W_a2, b_a2):
    global _NC_CACHE
    f32 = np.float32

    reps = np.asarray(reps, f32)
    separate_imgs = np.asarray(separate_imgs, f32)
    visual_context = np.asarray(visual_context, f32)
    hist = np.asarray(hist, f32)
    hist_len = np.asarray(hist_len, np.int32)
    masks = np.asarray(masks)[:, :, 0]          # [B, L] True = masked

    # ---- host mask compaction: gather valid columns of reps.T ----
    reps8 = np.zeros((B, P, KE * LC), bf16)
    mask_row = np.full((B, LC), f32(-1e30))
    for i in range(B):
        idx = np.nonzero(~masks[i])[0]
        n = len(idx)
        assert n <= LC, f"valid count {n} exceeds LC={LC}"
        rT = reps[i][idx].T                      # [EMBED, n]
        pad = np.zeros((EMBED, LC), f32)
        pad[:, :n] = rT
        reps8[i] = (pad.reshape(KE, P, LC).transpose(1, 0, 2)
                    .reshape(P, KE * LC).astype(bf16))
        mask_row[i, :n] = 0.0
    mask_row += f32(b_a2[0])

    constP = np.zeros((P, CPW), f32)
    constP[:, 0:P] = np.eye(P, dtype=f32)
    constP[:, P:P + NHT] = np.asarray(b_emb, f32).reshape(NHT, P).T
    constP[:, P + NHT:P + 2 * NHT] = np.asarray(b_vis, f32).reshape(NHT, P).T
    constP[:, P + 2 * NHT:P + 3 * NHT] = np.asarray(b_mm, f32).reshape(NHT, P).T
    constP[:, P + 3 * NHT:P + 3 * NHT + NAT] = (
        np.asarray(b_a1, f32).reshape(NAT, P).T)

    const1b_shared = np.zeros((1, C1BW), bf16)
    const1b_shared[0, 0:P] = 1.0
    const1b_shared[0, P:P + HID] = np.asarray(b_sep, f32).astype(bf16)
    const1b_shared[0, P + HID:P + 2 * HID] = np.asarray(b_emb, f32).astype(bf16)

    wvis8 = _packT(W_vis).reshape(P, NVG, KVG * HID).transpose(1, 0, 2)
    wvis8 = np.ascontiguousarray(wvis8)

    shared = {
        "wemb8": _packT(W_emb),
        "wmm8": _packT(W_mm),
        "wa18": _packT(W_a1),
        "wsep8": _packT(W_sep),
        "wvis8": wvis8,
        "wa28": np.ascontiguousarray(
            np.asarray(W_a2, f32).reshape(KA, P).T).astype(bf16),
        "constP": constP,
    }

    in_maps = []
    for c in range(NCORES):
        sl = slice(c * BC, (c + 1) * BC)
        hl = hist_len[sl].reshape(BS)
        hvalid = (np.arange(H)[None, :] < hl[:, None]).astype(f32)
        hvalid /= np.maximum(hl, 1).astype(f32)[:, None]
        validW = np.zeros((BSH, BS), f32)
        for bs in range(BS):
            validW[bs * H:(bs + 1) * H, bs] = hvalid[bs]
        const1b = const1b_shared.copy()
        const1b[0, P + 2 * HID:] = mask_row[sl].reshape(-1).astype(bf16)
        m = {
            "reps8": reps8[sl],
            "vct8": _packT(np.ascontiguousarray(visual_context[sl].T)),
            "sepT8": _packT(
                np.ascontiguousarray(separate_imgs[sl].reshape(BS, IMG).T)),
            "histf8": _packT(hist[sl].reshape(BSH, EMBED)),
            "validW8": _packT(validW),
            "const1b": const1b,
            "hh_col": (hl > 0).astype(f32).reshape(BS, 1),
        }
        m.update(shared)
        in_maps.append(m)

    if _NC_CACHE is None:
        _NC_CACHE = build_nc()
    res = run_bass_kernel_spmd(_NC_CACHE, in_maps, list(range(NCORES)))
    # out dram is [S, BC] per core: out[s, b] -> [BC, S, 1]
    out = np.concatenate(
        [r["out"].T.reshape(BC, S, 1) for r in res.results], axis=0)
    return out.astype(f32)


if __name__ == "__main__":
    pass


# revision 50
# speedup vs baseline: 1.0464x; 1.0464x over previous
"""Trainium2 Bass kernel for nn_ListenerModel (scatter_memory).

Data-parallel over batch (B=64 -> 8 rows/core):
 - bf16 matmul operands everywhere (half the HBM bytes of fp32/fp32r at
   the same PE rate, no small-free-dim penalty).
 - Host-side compaction of masked softmax positions: ~50% of L=512 get
   weight exactly 0, so the mm1->mm2->att chain runs on LC=288 columns.
 - W_vis stays replicated (a ReduceScatter variant was tried: the
   collective runway is ~65us from kernel start and couples the span to
   inter-core launch skew, which is both slower in expectation and high
   variance).  The 12.6MB bf16 stream is consumed chunk-by-chunk by the
   PE, interleaved with mm1 so the array never waits for it.
 - Partition-major packed DRAM layouts (multi-KB DMA descriptor lines),
   latency-ordered on the Sync queue; tiny constants on GpSimd.
 - Softmax without max-subtraction (scores are tanh-bounded, exp is
   fp32-safe); the mask row (carrying b_a2) is accumulated into the
   score matmul as a rank-1 PE term; softmax scale on DVE.
 - The softmax -> broadcast -> weighted-sum tail is software-pipelined
   one batch row behind the mm2/mm3 chain, and the final
   sep_fin . attended dot is a per-row PE matmul (sepfinT x attT col).
"""

import numpy as np
import ml_dtypes
from contextlib import ExitStack

import concourse.bass as bass
import concourse.mybir as mybir
from concourse import bacc, tile
from concourse.bass_utils import run_bass_kernel_spmd

NCORES = 8
B, L, S, H = 64, 512, 6, 8
EMBED, HID, IMG, ATT = 1024, 512, 2048, 256
SIMG = S * IMG          # 12288
BC = B // NCORES        # 8 batch rows per core
BS = BC * S             # 48 (b,s) rows per core
BSH = BS * H            # 384
P = 128
LC = 288                # compacted sequence length (max valid ~284)
FP = mybir.dt.float32
FPR = mybir.dt.float32r
BF = mybir.dt.bfloat16

KE = EMBED // P         # 8  k-chunks for EMBED contraction
KH = HID // P           # 4  k-chunks for HID contraction
KA = ATT // P           # 2  k-chunks for ATT contraction
KI = IMG // P           # 16 k-chunks for separate-image projection
KBH = BSH // P          # 3  k-chunks for history averaging
KV = SIMG // P          # 96 k-chunks for the visual-context matmul
NVG = 12                # W_vis DMA groups
KVG = KV // NVG         # 8 k-chunks per group
NHT = HID // P          # 4  hid tiles
NAT = ATT // P          # 2  att tiles

# packed [P, *] fp32 constants: ident | bembc | bvisc | bmmc | ba1c
CPW = P + NHT * 3 + NAT
# packed [1, *] bf16: ones_bf | bsep_row | bemb_row | mask rows
C1BW = P + HID + HID + BC * LC

bf16 = ml_dtypes.bfloat16


def build_nc():
    nc = bacc.Bacc(None)

    d_constP = nc.dram_tensor("constP", [P, CPW], FP, kind="ExternalInput")
    d_const1b = nc.dram_tensor("const1b", [1, C1BW], BF, kind="ExternalInput")
    d_reps = nc.dram_tensor("reps8", [BC, P, KE * LC], BF, kind="ExternalInput")
    d_wvis = nc.dram_tensor("wvis8", [NVG, P, KVG * HID], BF,
                            kind="ExternalInput")
    d_vct = nc.dram_tensor("vct8", [P, KV * BC], BF, kind="ExternalInput")
    d_wemb = nc.dram_tensor("wemb8", [P, KE * HID], BF, kind="ExternalInput")
    d_wmm = nc.dram_tensor("wmm8", [P, 2 * KH * HID], BF, kind="ExternalInput")
    d_wa1 = nc.dram_tensor("wa18", [P, KH * ATT], BF, kind="ExternalInput")
    d_wsep = nc.dram_tensor("wsep8", [P, KI * HID], BF, kind="ExternalInput")
    d_wa2 = nc.dram_tensor("wa28", [P, KA], BF, kind="ExternalInput")
    d_sepT = nc.dram_tensor("sepT8", [P, KI * BS], BF, kind="ExternalInput")
    d_hist = nc.dram_tensor("histf8", [P, KBH * EMBED], BF, kind="ExternalInput")
    d_validW = nc.dram_tensor("validW8", [P, KBH * BS], BF, kind="ExternalInput")
    d_hh = nc.dram_tensor("hh_col", [BS, 1], FP, kind="ExternalInput")
    d_out = nc.dram_tensor("out", [S, BC], FP, kind="ExternalOutput")

    AFT = mybir.ActivationFunctionType
    AX = mybir.AxisListType

    with ExitStack() as ctx:
        tc = ctx.enter_context(tile.TileContext(nc))
        wres = ctx.enter_context(tc.tile_pool(name="wres", bufs=1))
        wvp = ctx.enter_context(tc.tile_pool(name="wvp", bufs=3))
        repsp = ctx.enter_context(tc.tile_pool(name="repsp", bufs=8))
        mm1p = ctx.enter_context(tc.tile_pool(name="mm1p", bufs=32))
        mm2p = ctx.enter_context(tc.tile_pool(name="mm2p", bufs=8))
        atthp = ctx.enter_context(tc.tile_pool(name="atthp", bufs=4))
        tmpp = ctx.enter_context(tc.tile_pool(name="tmpp", bufs=2))
        smp = ctx.enter_context(tc.tile_pool(name="smp", bufs=4))
        psA = ctx.enter_context(tc.tile_pool(name="psA", bufs=5, space="PSUM"))
        psB = ctx.enter_context(tc.tile_pool(name="psB", bufs=2, space="PSUM"))
        psD = ctx.enter_context(tc.tile_pool(name="psD", bufs=1, space="PSUM"))

        def wtile(shape, tag, dt=FP):
            return wres.tile(shape, dt, tag=tag, name=tag)

        def body():
            # ---- Sync queue: big loads in latency-priority order ----
            vct = wtile([P, KV * BC], "vct", BF)
            nc.sync.dma_start(out=vct, in_=d_vct[:, :])
            wv_loads = []  # (tile, n_chunks)

            def load_wv(g, c0=0, cn=KVG):
                t = wvp.tile([P, cn * HID], BF, tag="wv", name=f"wv{g}_{c0}")
                nc.sync.dma_start(out=t, in_=d_wvis[g][:, c0 * HID:(c0 + cn) * HID])
                wv_loads.append((t, cn))

            rt = []

            def load_rt(b):
                t = repsp.tile([P, KE * LC], BF, tag="reps", name="rt")
                nc.sync.dma_start(out=t, in_=d_reps[b])
                rt.append(t)

            load_wv(0)
            wemb = wtile([P, KE * HID], "wemb", BF)
            nc.sync.dma_start(out=wemb, in_=d_wemb[:, :])
            load_rt(0)
            load_wv(1)
            load_rt(1)
            load_wv(2)
            load_rt(2)
            load_rt(3)
            load_wv(3)
            for b in range(4, BC):
                load_rt(b)
            wmm = wtile([P, 2 * KH * HID], "wmm", BF)
            nc.sync.dma_start(out=wmm, in_=d_wmm[:, :])
            for g in range(4, NVG):
                load_wv(g)
            wsep = wtile([P, KI * HID], "wsep", BF)
            nc.sync.dma_start(out=wsep, in_=d_wsep[:, :])
            sepT = wtile([P, KI * BS], "sepT", BF)
            nc.sync.dma_start(out=sepT, in_=d_sepT[:, :])
            histf = wtile([P, KBH * EMBED], "histf", BF)
            nc.sync.dma_start(out=histf, in_=d_hist[:, :])
            validW = wtile([P, KBH * BS], "validW", BF)
            nc.sync.dma_start(out=validW, in_=d_validW[:, :])
            wa1 = wtile([P, KH * ATT], "wa1", BF)
            nc.sync.dma_start(out=wa1, in_=d_wa1[:, :])
            wa2_sb = wtile([P, KA], "wa2", BF)
            nc.sync.dma_start(out=wa2_sb, in_=d_wa2[:, :])

            # ---- GpSimd queue: tiny constants ----
            constP = wtile([P, CPW], "constP")
            nc.gpsimd.dma_start(out=constP, in_=d_constP[:, :])
            const1b = wtile([1, C1BW], "const1b", BF)
            nc.gpsimd.dma_start(out=const1b, in_=d_const1b[:, :])
            hh_sb = wtile([BS, 1], "hh")
            nc.gpsimd.dma_start(out=hh_sb, in_=d_hh[:, :])

            ident = constP[:, 0:P]
            bembc = constP[:, P:P + NHT]
            bvisc = constP[:, P + NHT:P + 2 * NHT]
            bmmc = constP[:, P + 2 * NHT:P + 3 * NHT]
            ba1c = constP[:, P + 3 * NHT:P + 3 * NHT + NAT]
            ones_bf = const1b[:, 0:P]
            bsep_row = const1b[:, P:P + HID]
            bemb_row = const1b[:, P + HID:P + 2 * HID]
            mrows = const1b[:, P + 2 * HID:]

            # ---- interleaved: vc chunk-groups + mm1 per batch row ----
            # vc_ps accumulates the whole 96-chunk contraction for this
            # core's 8 rows; mm1T[b] = relu(Wemb.T @ repsT[b] + b_emb)
            vc_ps = psB.tile([BC, HID], FP, tag="B", name="vc_ps")
            mm1_sb = {}

            kcur = 0

            def emit_vc_item(item):
                nonlocal kcur
                wv, cn = item
                for j in range(cn):
                    k = kcur + j
                    nc.tensor.matmul(vc_ps[:, :], vct[:, k * BC:(k + 1) * BC],
                                     wv[:, j * HID:(j + 1) * HID],
                                     start=(k == 0), stop=(k == KV - 1))
                kcur += cn

            def emit_mm1(b):
                for h in range(NHT):
                    ps = psA.tile([P, 512], FP, tag="A", name="mm1ps")
                    for k in range(KE):
                        nc.tensor.matmul(
                            ps[:, :LC],
                            wemb[:, k * HID + h * P:k * HID + (h + 1) * P],
                            rt[b][:, k * LC:(k + 1) * LC],
                            start=(k == 0), stop=(k == KE - 1))
                    t = mm1p.tile([P, LC], BF, tag="mm1", name=f"mm1_{b}_{h}")
                    nc.scalar.activation(t, ps[:, :LC], AFT.Relu,
                                         bias=bembc[:, h:h + 1])
                    mm1_sb[(b, h)] = t

            # ~12 wvis k-chunks per mm1 row keeps the vc stream fed
            idone = 0
            for b in range(BC):
                ktarget = (b + 1) * KV // BC
                while kcur < ktarget and idone < len(wv_loads):
                    emit_vc_item(wv_loads[idone])
                    idone += 1
                emit_mm1(b)
            while idone < len(wv_loads):
                emit_vc_item(wv_loads[idone])
                idone += 1

            # ctx: copy out of PSUM, transpose, relu(+bvis), ctxmm
            ctx_sb = wtile([BC, HID], "ctx_sb")
            nc.scalar.activation(ctx_sb, vc_ps[:, :], AFT.Identity)
            ctxT = []
            for h in range(NHT):
                tp = psB.tile([P, BC], FP, tag="B", name="ctxT_ps")
                nc.tensor.transpose(tp[:, :], ctx_sb[:, h * P:(h + 1) * P],
                                    ident[:BC, :BC])
                t = wtile([P, BC], f"ctxT{h}", BF)
                nc.scalar.activation(t, tp[:, :], AFT.Relu,
                                     bias=bvisc[:, h:h + 1])
                ctxT.append(t)
            ctxmmb = []
            for h2 in range(NHT):
                ps = psB.tile([P, BC], FP, tag="B", name="ctxmm_ps")
                for k in range(KH):
                    nc.tensor.matmul(
                        ps[:, :],
                        wmm[:, (KH + k) * HID + h2 * P:(KH + k) * HID + (h2 + 1) * P],
                        ctxT[k][:, :],
                        start=(k == 0), stop=(k == KH - 1))
                t = wtile([P, BC], f"ctxmmb{h2}")
                nc.scalar.activation(t, ps[:, :], AFT.Identity,
                                     bias=bmmc[:, h2:h2 + 1])
                ctxmmb.append(t)

            # ---- separate images projection: sep[48, 512] ----
            sep_ps = psB.tile([BS, HID], FP, tag="B", name="sep_ps")
            for k in range(KI):
                nc.tensor.matmul(sep_ps[:, :], sepT[:, k * BS:(k + 1) * BS],
                                 wsep[:, k * HID:(k + 1) * HID],
                                 start=(k == 0), stop=False)
            nc.tensor.matmul(sep_ps[:, :], ones_bf[:, :BS], bsep_row,
                             start=False, stop=True)
            sep_sb = wtile([BS, HID], "sep_sb")
            nc.vector.tensor_copy(sep_sb, sep_ps[:, :])

            # ---- history: havgT[e] = histf.T @ validW (pre-averaged) ----
            havgT = []
            for e in range(KE):
                ps = psB.tile([P, BS], FP, tag="B", name="havg_ps")
                for k in range(KBH):
                    nc.tensor.matmul(
                        ps[:, :],
                        histf[:, k * EMBED + e * P:k * EMBED + (e + 1) * P],
                        validW[:, k * BS:(k + 1) * BS],
                        start=(k == 0), stop=(k == KBH - 1))
                t = wtile([P, BS], f"havgT{e}", BF)
                nc.scalar.activation(t, ps[:, :], AFT.Identity)
                havgT.append(t)

            # hist_add[48, 512] = relu(havg @ W_emb + b_emb)
            ha_ps = psB.tile([BS, HID], FP, tag="B", name="ha_ps")
            for e in range(KE):
                nc.tensor.matmul(ha_ps[:, :], havgT[e][:, :],
                                 wemb[:, e * HID:(e + 1) * HID],
                                 start=(e == 0), stop=False)
            nc.tensor.matmul(ha_ps[:, :], ones_bf[:, :BS], bemb_row,
                             start=False, stop=True)
            hadd_sb = wtile([BS, HID], "hadd_sb")
            nc.scalar.activation(hadd_sb, ha_ps[:, :], AFT.Relu)

            # sep_final = sep + hh * hist_add, then transpose to [hid, 48]
            sepfin = wtile([BS, HID], "sepfin")
            nc.vector.tensor_scalar_mul(sepfin, hadd_sb, hh_sb)
            nc.vector.tensor_add(sepfin, sepfin, sep_sb)
            sepfinT = []
            for h in range(NHT):
                tp = psB.tile([P, BS], FP, tag="B", name="sft_ps")
                nc.tensor.transpose(tp[:, :], sepfin[:, h * P:(h + 1) * P],
                                    ident[:BS, :BS])
                t = wtile([P, BS], f"sepfinT{h}")
                nc.scalar.activation(t, tp[:, :], AFT.Identity)
                sepfinT.append(t)

            # ---- per-b chain, softmax/weighted-sum pipelined one b behind ----
            attT = [wtile([P, BC], f"attT{h}") for h in range(NHT)]
            out_sb = wtile([S, BC], "out_sb")
            pend = {}

            def emit_tail(bp):
                wrow, mm2t = pend.pop(bp)
                wb_ps = psA.tile([P, 512], FP, tag="A", name="wbps")
                nc.tensor.matmul(wb_ps[:, :LC], ones_bf, wrow[:, :],
                                 start=True, stop=True)
                for h2 in range(NHT):
                    tmp = tmpp.tile([P, LC], BF, tag="tmpa", name="tmpa")
                    nc.vector.tensor_mul(tmp, mm2t[h2][:, :], wb_ps[:, :LC])
                    nc.vector.reduce_sum(attT[h2][:, bp:bp + 1], tmp,
                                         axis=AX.X)

            def emit_dot(bp):
                # out[6] for this b: sepfinT chunks . attT column (PE dot)
                dps = psD.tile([S, 1], FP, tag="D", name="dot_ps")
                for h in range(NHT):
                    nc.tensor.matmul(dps[:, :],
                                     sepfinT[h][:, bp * S:(bp + 1) * S],
                                     attT[h][:, bp:bp + 1],
                                     start=(h == 0), stop=(h == NHT - 1))
                nc.vector.tensor_copy(out_sb[:, bp:bp + 1], dps[:, :])

            for b in range(BC):
                # mm2 = relu(Wmm_top.T @ mm1 + ctxmm[:, b]) -> bf16.
                # finalize split DVE/ACT via the per-partition ctx bias.
                mm2t = []
                for h2 in range(NHT):
                    ps = psA.tile([P, 512], FP, tag="A", name="mm2ps")
                    for k in range(KH):
                        nc.tensor.matmul(
                            ps[:, :LC],
                            wmm[:, k * HID + h2 * P:k * HID + (h2 + 1) * P],
                            mm1_sb[(b, k)][:, :],
                            start=(k == 0), stop=(k == KH - 1))
                    t = mm2p.tile([P, LC], BF, tag="mm2", name="mm2t")
                    if h2 < 2:
                        nc.vector.tensor_scalar(
                            out=t, in0=ps[:, :LC],
                            scalar1=ctxmmb[h2][:, b:b + 1], scalar2=0.0,
                            op0=mybir.AluOpType.add, op1=mybir.AluOpType.max)
                    else:
                        nc.scalar.activation(t, ps[:, :LC], AFT.Relu,
                                             bias=ctxmmb[h2][:, b:b + 1])
                    mm2t.append(t)
                # mm3: atthT = tanh(W_a1.T @ mm2T + b_a1)
                atth = []
                for a in range(NAT):
                    ps = psA.tile([P, 512], FP, tag="A", name="mm3ps")
                    for k in range(KH):
                        nc.tensor.matmul(
                            ps[:, :LC],
                            wa1[:, k * ATT + a * P:k * ATT + (a + 1) * P],
                            mm2t[k][:, :],
                            start=(k == 0), stop=(k == KH - 1))
                    t = atthp.tile([P, LC], BF, tag="atth", name="atht")
                    nc.scalar.activation(t, ps[:, :LC], AFT.Tanh,
                                         bias=ba1c[:, a:a + 1])
                    atth.append(t)
                # scores row [1, LC] = W_a2.T @ atthT; the mask row (which
                # carries b_a2 too) is accumulated as a rank-1 PE term
                sc_ps = psB.tile([1, 512], FP, tag="B", name="scps")
                for k in range(KA):
                    nc.tensor.matmul(sc_ps[:, :LC], wa2_sb[:, k:k + 1],
                                     atth[k][:, :],
                                     start=(k == 0), stop=False)
                nc.tensor.matmul(sc_ps[:, :LC], ones_bf[:, 0:1],
                                 mrows[:, b * LC:(b + 1) * LC],
                                 start=False, stop=True)
                # softmax without max-subtraction: scores are tanh-bounded
                # (|s| <= 25.6), exp stays comfortably inside fp32
                att_row = smp.tile([1, LC], FP, tag="attrow", name="att_row")
                esum = smp.tile([1, 1], FP, tag="esum", name="esum")
                nc.scalar.activation(att_row, sc_ps[:, :LC], AFT.Exp,
                                     accum_out=esum)
                rec = smp.tile([1, 1], FP, tag="rec", name="rec")
                nc.vector.reciprocal(rec, esum)
                wrow = smp.tile([1, LC], BF, tag="wrow", name="wrow")
                nc.vector.tensor_scalar_mul(wrow, att_row, rec)
                pend[b] = (wrow, mm2t)
                if b > 0:
                    emit_tail(b - 1)
                if b > 1:
                    emit_dot(b - 2)
            emit_tail(BC - 1)
            emit_dot(BC - 2)
            emit_dot(BC - 1)

            nc.sync.dma_start(out=d_out[:, :], in_=out_sb)

        body()

    nc.compile()
    return nc


def _packT(a, np_dt=bf16):
    """[K, W] fp32 -> [P, (K//P)*W] partition-major packed."""
    a = np.ascontiguousarray(a, np.float32)
    K, W = a.shape
    return np.ascontiguousarray(
        a.reshape(K // P, P, W).transpose(1, 0, 2).reshape(P, (K // P) * W)
    ).astype(np_dt)


_NC_CACHE = None


def kernel(reps, separate_imgs, visual_context, masks, hist, hist_len,
           W_vis, b_vis, W_emb, b_emb, W_mm, b_mm, W_sep, b_sep,
           W_a1, b_a1, W_a2, b_a2):
    global _NC_CACHE
    f32 = np.float32

    reps = np.asarray(reps, f32)
    separate_imgs = np.asarray(separate_imgs, f32)
    visual_context = np.asarray(visual_context, f32)
    hist = np.asarray(hist, f32)
    hist_len = np.asarray(hist_len, np.int32)
    masks = np.asarray(masks)[:, :, 0]          # [B, L] True = masked

    # ---- host mask compaction: gather valid columns of reps.T ----
    reps8 = np.zeros((B, P, KE * LC), bf16)
    mask_row = np.full((B, LC), f32(-1e30))
    for i in range(B):
        idx = np.nonzero(~masks[i])[0]
        n = len(idx)
        assert n <= LC, f"valid count {n} exceeds LC={LC}"
        rT = reps[i][idx].T                      # [EMBED, n]
        pad = np.zeros((EMBED, LC), f32)
        pad[:, :n] = rT
        reps8[i] = (pad.reshape(KE, P, LC).transpose(1, 0, 2)
                    .reshape(P, KE * LC).astype(bf16))
        mask_row[i, :n] = 0.0
    mask_row += f32(b_a2[0])

    constP = np.zeros((P, CPW), f32)
    constP[:, 0:P] = np.eye(P, dtype=f32)
    constP[:, P:P + NHT] = np.asarray(b_emb, f32).reshape(NHT, P).T
    constP[:, P + NHT:P + 2 * NHT] = np.asarray(b_vis, f32).reshape(NHT, P).T
    constP[:, P + 2 * NHT:P + 3 * NHT] = np.asarray(b_mm, f32).reshape(NHT, P).T
    constP[:, P + 3 * NHT:P + 3 * NHT + NAT] = (
        np.asarray(b_a1, f32).reshape(NAT, P).T)

    const1b_shared = np.zeros((1, C1BW), bf16)
    const1b_shared[0, 0:P] = 1.0
    const1b_shared[0, P:P + HID] = np.asarray(b_sep, f32).astype(bf16)
    const1b_shared[0, P + HID:P + 2 * HID] = np.asarray(b_emb, f32).astype(bf16)

    wvis8 = _packT(W_vis).reshape(P, NVG, KVG * HID).transpose(1, 0, 2)
    wvis8 = np.ascontiguousarray(wvis8)

    shared = {
        "wemb8": _packT(W_emb),
        "wmm8": _packT(W_mm),
        "wa18": _packT(W_a1),
        "wsep8": _packT(W_sep),
        "wvis8": wvis8,
        "wa28": np.ascontiguousarray(
            np.asarray(W_a2, f32).reshape(KA, P).T).astype(bf16),
        "constP": constP,
    }

    in_maps = []
    for c in range(NCORES):
        sl = slice(c * BC, (c + 1) * BC)
        hl = hist_len[sl].reshape(BS)
        hvalid = (np.arange(H)[None, :] < hl[:, None]).astype(f32)
        hvalid /= np.maximum(hl, 1).astype(f32)[:, None]
        validW = np.zeros((BSH, BS), f32)
        for bs in range(BS):
            validW[bs * H:(bs + 1) * H, bs] = hvalid[bs]
        const1b = const1b_shared.copy()
        const1b[0, P + 2 * HID:] = mask_row[sl].reshape(-1).astype(bf16)
        m = {
            "reps8": reps8[sl],
            "vct8": _packT(np.ascontiguousarray(visual_context[sl].T)),
            "sepT8": _packT(
                np.ascontiguousarray(separate_imgs[sl].reshape(BS, IMG).T)),
            "histf8": _packT(hist[sl].reshape(BSH, EMBED)),
            "validW8": _packT(validW),
            "const1b": const1b,
            "hh_col": (hl > 0).astype(f32).reshape(BS, 1),
        }
        m.update(shared)
        in_maps.append(m)

    if _NC_CACHE is None:
        _NC_CACHE = build_nc()
    res = run_bass_kernel_spmd(_NC_CACHE, in_maps, list(range(NCORES)))
    # out dram is [S, BC] per core: out[s, b] -> [BC, S, 1]
    out = np.concatenate(
        [r["out"].T.reshape(BC, S, 1) for r in res.results], axis=0)
    return out.astype(f32)


if __name__ == "__main__":
    pass
